# revision 1
# baseline (speedup 1.0000x reference)
"""Trainium2 Bass kernel for nn_ModelDEP (biaffine-ish dependency parser loss).

Contract: kernel(**inputs) takes FULL unsharded numpy inputs (as produced by
reference.setup_inputs()) and returns the FULL output (scalar f32 loss).

Strategy (hardcoded, self-contained):
  - Data parallel over batch: B=16 examples -> 8 cores x 2 examples.
  - Per example, on device:
      hidden_T = relu(W1.T @ ctx_T + b1)            [256h x 128i] (h on partitions)
      cwr_T    = [root | hidden_T]                  [256h x 129j]
      ha_T     = Wa.T @ hidden_T + bp               [256 x 128]   (bp folded here)
      cbb_T    = Wb.T @ cwr_T                       [256 x 129]
      arc[i,j] = W_arc . relu(ha_T[:,i] + cbb_T[:,j])
        - per (j, h-chunk): one fused (add bias, max 0) op -> bf16 [128,128] tile
          (split between DVE tensor_scalar and ACT activation-relu-with-bias)
        - TensorE: lhsT = pairs tile (stationary, bf16 FWL), rhs = W_arc chunk
          -> PSUM column [128i, 1], accumulated over the 2 h-chunks
      arc CE via logsumexp over j (reduce_max / exp+accum / ln) and gold logit
        via (iota == gold) * logits with fused accumulate.
      label path: cbb rows in [j,h] layout -> internal DRAM; indirect-DMA gather
        rows at gold arcs; PE transpose; sel_T = relu(ha_T + gathered.T);
        label logits = sel_T.T @ W_lab + b_lab; logsumexp + gold as above.
      per-token ce = arc_ce + lab_ce -> [128, 2] output per core.
  - Host: mask by sentence length, global sum, /denom, *0.5.
"""

import sys
import numpy as np

for _p in ("/opt/trn_rl_repo", "/root/.axon_site/_ro/trn_rl_repo"):
    if _p not in sys.path:
        sys.path.append(_p)

import ml_dtypes

import concourse.bass as bass
from concourse import bacc
import concourse.mybir as mybir
import concourse.tile as tile
from concourse.bass import IndirectOffsetOnAxis
from concourse.bass_utils import run_bass_kernel_spmd
from concourse.masks import make_identity
from concourse.tile_rust import add_dep_helper

BF16 = mybir.dt.bfloat16
FP8 = mybir.dt.float8e4
F32 = mybir.dt.float32
I32 = mybir.dt.int32
AF = mybir.ActivationFunctionType
ALU = mybir.AluOpType

B, L, D, H, TAGS = 16, 128, 512, 256, 45
NC_CORES = 8
NB = B // NC_CORES  # examples per core
J = L + 1  # head candidates (root + tokens)
HC = H // 128  # h chunks
DC = D // 128  # d chunks

_nb = ml_dtypes.bfloat16

_cached = {}

# j-loop relu engine split by (j*HC+hc) % 10: measured rates
# DVE ~163ns, GPSIMD ~?ns, ACT ~326ns per [128,128] tile
# GPSIMD shares SBUF ports with DVE - do NOT give it elementwise work.
# DVE rate ~162ns/tile, ACT ~316ns/tile -> ~2:1 split over k%20.
RELU_ACT = {2, 5, 8, 11, 14, 17}


def _build_program():
    nc = bacc.Bacc("TRN2", target_bir_lowering=False, debug=False, num_devices=NC_CORES)

    # ---- I/O ----
    ctx_d = nc.dram_tensor("ctx_bf", [NB, 128, DC, 128], BF16, kind="ExternalInput")
    w1_d = nc.dram_tensor("w1_bf", [128, DC, H], BF16, kind="ExternalInput")
    wa_d = nc.dram_tensor("wa_bf", [128, HC, H], BF16, kind="ExternalInput")
    wb_d = nc.dram_tensor("wb_bf", [128, HC, H], BF16, kind="ExternalInput")
    pkf_d = nc.dram_tensor("pack_f32", [128, 310], F32, kind="ExternalInput")
    pkb_d = nc.dram_tensor("pack_bf", [128, 4 + HC * TAGS], BF16, kind="ExternalInput")
    gidx_d = nc.dram_tensor("gidx_i", [128, NB], I32, kind="ExternalInput")
    ce_d = nc.dram_tensor("ce_out", [128, NB], F32, kind="ExternalOutput")
    cbb_ds = [nc.dram_tensor(f"cbb_scratch{b}", [J, H], F32) for b in range(NB)]

    with tile.TileContext(nc) as tc:
        with (
            tc.tile_pool(name="consts", bufs=1) as consts,
            tc.tile_pool(name="bpool", bufs=2) as bpool,
            tc.tile_pool(name="pairs", bufs=48) as pairs_pool,
            tc.tile_pool(name="ps_big", bufs=2, space="PSUM") as ps_big,
            tc.tile_pool(name="ps_work", bufs=2, space="PSUM") as ps_work,
            tc.tile_pool(name="ps_lab", bufs=2, space="PSUM") as ps_lab,
        ):
            # identity build first (gpsimd is otherwise idle here)
            ident_sb = consts.tile([128, 128], F32)
            make_identity(nc, ident_sb[:])
            # ---- ctx for both batches first (critical path) ----
            ctxTs = []
            ctxT0 = bpool.tile([128, DC, 128], BF16, tag="ctxT")
            nc.sync.dma_start(out=ctxT0[:, 0:2, :], in_=ctx_d.ap()[0, :, 0:2, :])
            nc.scalar.dma_start(out=ctxT0[:, 2:4, :], in_=ctx_d.ap()[0, :, 2:4, :])
            ctxTs.append(ctxT0)
            # ---- load constants: 2 packed DMAs + 3 big weights ----
            w1_sb = consts.tile([128, DC, H], BF16)
            nc.gpsimd.dma_start(out=w1_sb[:], in_=ctx_ap(w1_d))
            ctxT1 = bpool.tile([128, DC, 128], BF16, tag="ctxT")
            nc.sync.dma_start(out=ctxT1[:], in_=ctx_d.ap()[1])
            ctxTs.append(ctxT1)
            wa_sb = consts.tile([128, HC, H], BF16)
            nc.scalar.dma_start(out=wa_sb[:], in_=ctx_ap(wa_d))
            wb_sb = consts.tile([128, HC, H], BF16)
            nc.sync.dma_start(out=wb_sb[:], in_=ctx_ap(wb_d))
            pkf_sb = consts.tile([128, 310], F32)
            nc.sync.dma_start(out=pkf_sb[:], in_=ctx_ap(pkf_d))
            pkb_sb = consts.tile([128, 4 + HC * TAGS], BF16)
            nc.gpsimd.dma_start(out=pkb_sb[:], in_=ctx_ap(pkb_d))
            gidx_sb = consts.tile([128, NB], I32)
            nc.gpsimd.dma_start(out=gidx_sb[:], in_=ctx_ap(gidx_d))
            ce_sb = consts.tile([128, NB], F32)

            arc_pss = []
            lab_pss = []
            for b in range(NB):
                ctxT = ctxTs[b]
                # ---- hidden (into cwr cols 1..128) ----
                cwrT = bpool.tile([128, HC, J], BF16, tag="cwrT")
                for hc in range(HC):
                    nc.vector.tensor_copy(cwrT[:, hc, 0:1], pkb_sb[:, hc : hc + 1])
                for hc in range(HC):
                    phw = ps_work.tile([128, H], F32, tag="work")
                    ph = phw[:, :128]
                    for dc in range(DC):
                        nc.tensor.matmul(
                            ph[:],
                            lhsT=w1_sb[:, dc, hc * 128 : (hc + 1) * 128],
                            rhs=ctxT[:, dc, :],
                            start=(dc == 0),
                            stop=(dc == DC - 1),
                        )
                    nc.vector.tensor_scalar(
                        out=cwrT[:, hc, 1:J],
                        in0=ph[:],
                        scalar1=pkf_sb[:, hc : hc + 1],
                        scalar2=0.0,
                        op0=ALU.add,
                        op1=ALU.max,
                    )
                # ---- ha_T (+bp folded) ----
                haT = bpool.tile([128, HC, 128], BF16, tag="haT")
                for ac in range(HC):
                    paw = ps_work.tile([128, H], F32, tag="work")
                    pa = paw[:, :128]
                    for hc in range(HC):
                        nc.tensor.matmul(
                            pa[:],
                            lhsT=wa_sb[:, hc, ac * 128 : (ac + 1) * 128],
                            rhs=cwrT[:, hc, 1:J],
                            start=(hc == 0),
                            stop=(hc == HC - 1),
                        )
                    nc.vector.tensor_scalar(
                        out=haT[:, ac, :],
                        in0=pa[:],
                        scalar1=pkf_sb[:, 2 + ac : 3 + ac],
                        scalar2=None,
                        op0=ALU.add,
                    )
                # ---- cbb_T [128, 2, 129] f32 ----
                cbbT = bpool.tile([128, HC, J], F32, tag="cbbT")
                for bc in range(HC):
                    pc = ps_big.tile([128, J], F32, tag="pcb")
                    for hc in range(HC):
                        nc.tensor.matmul(
                            pc[:],
                            lhsT=wb_sb[:, hc, bc * 128 : (bc + 1) * 128],
                            rhs=cwrT[:, hc, :],
                            start=(hc == 0),
                            stop=(hc == HC - 1),
                        )
                    nc.scalar.copy(cbbT[:, bc, :], pc[:])
                # ---- cbb in [j, h] layout -> DRAM (for the gather) ----
                cj = bpool.tile([128, H], F32, tag="cj")
                pj = ps_work.tile([128, H], F32, tag="work")
                for hc in range(HC):
                    nc.tensor.matmul(
                        pj[:],
                        lhsT=cwrT[:, hc, 0:128],
                        rhs=wb_sb[:, hc, :],
                        start=(hc == 0),
                        stop=(hc == HC - 1),
                    )
                nc.scalar.copy(cj[:], pj[:])
                st1 = nc.sync.dma_start(
                    out=cbb_ds[b].ap()[0:128, :], in_=cj[:]
                )
                cjl = bpool.tile([1, H], F32, tag="cjl")
                pjlw = ps_work.tile([128, H], F32, tag="work")
                pjl = pjlw[0:1, :]
                for hc in range(HC):
                    nc.tensor.matmul(
                        pjl[:],
                        lhsT=cwrT[:, hc, 128:129],
                        rhs=wb_sb[:, hc, :],
                        start=(hc == 0),
                        stop=(hc == HC - 1),
                    )
                nc.scalar.copy(cjl[:], pjl[:])
                st2 = nc.sync.dma_start(
                    out=cbb_ds[b].ap()[128:J, :], in_=cjl[:]
                )
                # ---- gather cbb rows at gold arcs ----
                csel = bpool.tile([128, H], F32, tag="csel")
                g = nc.gpsimd.indirect_dma_start(
                    out=csel[:],
                    out_offset=None,
                    in_=cbb_ds[b].ap(),
                    in_offset=IndirectOffsetOnAxis(ap=gidx_sb[:, b : b + 1], axis=0),
                )
                add_dep_helper(g.ins, st1.ins, sync=True, reason="cbb store->gather")
                add_dep_helper(g.ins, st2.ins, sync=True, reason="cbb store->gather")

                # ---- label path ----
                selT = bpool.tile([128, HC, 128], BF16, tag="selT")
                for hc in range(HC):
                    ptrw = ps_work.tile([128, H], F32, tag="work")
                    ptr = ptrw[:, :128]
                    nc.tensor.transpose(
                        ptr[:], csel[:, hc * 128 : (hc + 1) * 128], ident_sb[:]
                    )
                    tmp = bpool.tile([128, 128], F32, tag="seltmp")
                    nc.vector.tensor_add(tmp[:], ptr[:], haT[:, hc, :])
                    nc.vector.tensor_scalar(
                        out=selT[:, hc, :], in0=tmp[:], scalar1=0.0, op0=ALU.max,
                        scalar2=None,
                    )
                lab_ps = ps_lab.tile([128, TAGS], F32, tag="lab")
                for hc in range(HC):
                    nc.tensor.matmul(
                        lab_ps[:],
                        lhsT=selT[:, hc, :],
                        rhs=pkb_sb[:, 4 + TAGS * hc : 4 + TAGS * (hc + 1)],
                        start=(hc == 0),
                        stop=False,
                    )
                nc.tensor.matmul(
                    lab_ps[:], lhsT=pkf_sb[0:1, 182:310], rhs=pkf_sb[0:1, 137 : 137 + TAGS], start=False, stop=True
                )
                lab_pss.append(lab_ps)

                # ---- the quadratic j-loop ----
                arc_ps = ps_big.tile([128, J], F32, tag="arc")
                for j in range(J):
                    for hc in range(HC):
                        pt = pairs_pool.tile([128, 128], BF16, tag="pairs")
                        k = (j * HC + hc) % 20
                        if k in RELU_ACT:
                            nc.scalar.activation(
                                pt[:],
                                haT[:, hc, :],
                                AF.Relu,
                                bias=cbbT[:, hc, j : j + 1],
                            )
                        else:
                            nc.vector.tensor_scalar(
                                out=pt[:],
                                in0=haT[:, hc, :],
                                scalar1=cbbT[:, hc, j : j + 1],
                                scalar2=0.0,
                                op0=ALU.add,
                                op1=ALU.max,
                            )
                        nc.tensor.matmul(
                            arc_ps[:, j : j + 1],
                            lhsT=pt[:],
                            rhs=pkb_sb[:, 2 + hc : 3 + hc],
                            start=(hc == 0),
                            stop=(hc == HC - 1),
                        )

                arc_pss.append(arc_ps)

            negms, negmls, ess, esls = [], [], [], []
            for b in range(NB):
                negm = bpool.tile([128, 1], F32, tag="negm")
                nc.vector.tensor_reduce(
                    negm[:], arc_pss[b][:], axis=mybir.AxisListType.X, op=ALU.max,
                    negate=True,
                )
                negms.append(negm)
                negml = bpool.tile([128, 1], F32, tag="negml")
                nc.vector.tensor_reduce(
                    negml[:], lab_pss[b][:], axis=mybir.AxisListType.X, op=ALU.max,
                    negate=True,
                )
                negmls.append(negml)
            for b in range(NB):
                et = bpool.tile([128, J], F32, tag="et")
                es = bpool.tile([128, 1], F32, tag="es")
                nc.scalar.activation(
                    et[:], arc_pss[b][:], AF.Exp, bias=negms[b][:], accum_out=es[:]
                )
                ess.append(es)
                etl = bpool.tile([128, TAGS], F32, tag="etl")
                esl = bpool.tile([128, 1], F32, tag="esl")
                nc.scalar.activation(
                    etl[:], lab_pss[b][:], AF.Exp, bias=negmls[b][:], accum_out=esl[:]
                )
                esls.append(esl)
            for b in range(NB):
                lns = bpool.tile([128, 1], F32, tag="lns")
                nc.scalar.activation(lns[:], ess[b][:], AF.Ln)
                lnsl = bpool.tile([128, 1], F32, tag="lnsl")
                nc.scalar.activation(lnsl[:], esls[b][:], AF.Ln)
                golda = bpool.tile([128, 1], F32, tag="golda")
                sc2 = bpool.tile([128, J], F32, tag="sc2")
                nc.vector.scalar_tensor_tensor(
                    out=sc2[:],
                    in0=pkf_sb[:, 8 : 8 + J],
                    scalar=pkf_sb[:, 4 + b : 5 + b],
                    op0=ALU.is_equal,
                    in1=arc_pss[b][:],
                    op1=ALU.mult,
                    accum_out=golda[:],
                )
                goldl = bpool.tile([128, 1], F32, tag="goldl")
                sc2l = bpool.tile([128, TAGS], F32, tag="sc2l")
                nc.vector.scalar_tensor_tensor(
                    out=sc2l[:],
                    in0=pkf_sb[:, 8 : 8 + TAGS],
                    scalar=pkf_sb[:, 6 + b : 7 + b],
                    op0=ALU.is_equal,
                    in1=lab_pss[b][:],
                    op1=ALU.mult,
                    accum_out=goldl[:],
                )
                cea = bpool.tile([128, 1], F32, tag="cea")
                nc.vector.tensor_sub(cea[:], lns[:], negms[b][:])
                nc.vector.tensor_sub(cea[:], cea[:], golda[:])
                cel = bpool.tile([128, 1], F32, tag="cel")
                nc.vector.tensor_sub(cel[:], lnsl[:], negmls[b][:])
                nc.vector.tensor_sub(cel[:], cel[:], goldl[:])
                nc.vector.tensor_add(ce_sb[:, b : b + 1], cea[:], cel[:])

            nc.sync.dma_start(out=ce_d.ap(), in_=ce_sb[:])

    nc.compile()
    return nc


def ctx_ap(d):
    return d.ap()


def _prep_in_maps(inputs):
    ctx = np.asarray(inputs["contextualized"], np.float32)
    arcs = np.asarray(inputs["desired_arcs"], np.int32)
    labs = np.asarray(inputs["desired_labels"], np.int32)
    W1 = np.asarray(inputs["W1"], np.float32)
    b1 = np.asarray(inputs["b1"], np.float32)
    root = np.asarray(inputs["root"], np.float32)
    Wp = np.asarray(inputs["Wp"], np.float32)
    bp = np.asarray(inputs["bp"], np.float32)
    W_arc = np.asarray(inputs["W_arc"], np.float32)
    W_lab = np.asarray(inputs["W_lab"], np.float32)
    b_lab = np.asarray(inputs["b_lab"], np.float32)

    def chunked(w, nch):  # [nch*128, X] -> [128, nch, X]
        return np.ascontiguousarray(
            w.reshape(nch, 128, -1).transpose(1, 0, 2)
        )

    w1_bf = chunked(W1, DC).astype(_nb)
    wa_bf = chunked(Wp[:H], HC).astype(_nb)
    wb_bf = chunked(Wp[H:], HC).astype(_nb)

    pkb = np.zeros((128, 4 + HC * TAGS), np.float32)
    pkb[:, 0:2] = root.reshape(HC, 128).T
    pkb[:, 2:4] = W_arc[:, 0].reshape(HC, 128).T
    for hc in range(HC):
        pkb[:, 4 + TAGS * hc : 4 + TAGS * (hc + 1)] = W_lab[hc * 128 : (hc + 1) * 128]
    pkb = pkb.astype(_nb)

    pkf_base = np.zeros((128, 310), np.float32)
    pkf_base[:, 0:2] = b1.reshape(HC, 128).T
    pkf_base[:, 2:4] = bp.reshape(HC, 128).T
    pkf_base[:, 8 : 8 + J] = np.arange(J, dtype=np.float32)[None, :]
    pkf_base[:, 137 : 137 + TAGS] = b_lab[None, :]
    pkf_base[:, 182:310] = 1.0

    in_maps = []
    for c in range(NC_CORES):
        bs = slice(c * NB, (c + 1) * NB)
        arcs_c = arcs[bs]  # [NB, 128]
        pkf = pkf_base.copy()
        pkf[:, 4:6] = arcs_c.T.astype(np.float32)
        pkf[:, 6:8] = labs[bs].T.astype(np.float32)
        in_maps.append(
            {
                "ctx_bf": np.ascontiguousarray(
                    ctx[bs].reshape(NB, L, DC, 128).transpose(0, 3, 2, 1)
                ).astype(_nb),
                "w1_bf": w1_bf,
                "wa_bf": wa_bf,
                "wb_bf": wb_bf,
                "pack_f32": pkf,
                "pack_bf": pkb,
                "gidx_i": np.ascontiguousarray(arcs_c.T).astype(np.int32),
            }
        )
    return in_maps


def kernel(**inputs) -> np.ndarray:
    if "nc" not in _cached:
        _cached["nc"] = _build_program()
    nc = _cached["nc"]
    in_maps = _prep_in_maps(inputs)
    res = run_bass_kernel_spmd(nc, in_maps, list(range(NC_CORES)))
    ce = np.concatenate([r["ce_out"] for r in res.results], axis=1)  # [128, B]
    lens = np.asarray(inputs["sentence_lengths"], np.int32)  # [B]
    mask = (np.arange(L)[None, :] < lens[:, None]).astype(np.float32)  # [B, L]
    total = float(np.sum(ce.T.astype(np.float64) * mask))
    denom = max(float(mask.sum()), 1.0)
    return np.array(0.5 * total / denom, dtype=np.float32)



# revision 3
# speedup vs baseline: 1.0550x; 1.0550x over previous
"""Trainium2 Bass kernel for nn_ModelDEP (biaffine-ish dependency parser loss).

Contract: kernel(**inputs) takes FULL unsharded numpy inputs (as produced by
reference.setup_inputs()) and returns the FULL output (scalar f32 loss).

Strategy (hardcoded, self-contained):
  - Data parallel over batch: B=16 examples -> 8 cores x 2 examples.
  - Per example, on device:
      hidden_T = relu(W1.T @ ctx_T + b1)            [256h x 128i] (h on partitions)
      cwr_T    = [root | hidden_T]                  [256h x 129j]
      ha_T     = Wa.T @ hidden_T + bp               [256 x 128]   (bp folded here)
      cbb_T    = Wb.T @ cwr_T                       [256 x 129]
      arc[i,j] = W_arc . relu(ha_T[:,i] + cbb_T[:,j])
        - per (j, h-chunk): one fused (add bias, max 0) op -> bf16 [128,128] tile
          split DVE tensor_scalar (163ns) : ACT activation relu+bias (292ns) = 9:5
        - TensorE: lhsT = pairs tile (stationary, bf16 FWL), rhs = W_arc chunk
          -> PSUM column [128i, 1], accumulated over the 2 h-chunks
      label path: cbb rows in [j,h] layout -> internal DRAM; indirect-DMA gather
        rows at gold arcs; PE transpose; sel_T = relu(ha_T + gathered.T);
        label logits = sel_T.T @ W_lab (no b_lab; host adds it).
      Ship raw logits: arc_ps [128,129] + lab_ps [128,45] f32 per example.
  - Host: log-softmax CE for arc+label, gold gather, mask by sentence length,
    global sum, /denom, *0.5.  (Softmax epilogue on host kills the Exp/Ln
    activation-table thrash and the on-chip reduce/exp/ln tail.)
"""

import sys
import numpy as np

for _p in ("/opt/trn_rl_repo", "/root/.axon_site/_ro/trn_rl_repo"):
    if _p not in sys.path:
        sys.path.append(_p)

import ml_dtypes

import concourse.bass as bass
from concourse import bacc
import concourse.mybir as mybir
import concourse.tile as tile
from concourse.bass import IndirectOffsetOnAxis
from concourse.bass_utils import run_bass_kernel_spmd
from concourse.masks import make_identity
from concourse.tile_rust import add_dep_helper

BF16 = mybir.dt.bfloat16
F32 = mybir.dt.float32
I32 = mybir.dt.int32
AF = mybir.ActivationFunctionType
ALU = mybir.AluOpType

B, L, D, H, TAGS = 16, 128, 512, 256, 45
NC_CORES = 8
NB = B // NC_CORES  # examples per core
J = L + 1  # head candidates (root + tokens)
HC = H // 128  # h chunks
DC = D // 128  # d chunks
OUTW = J + TAGS  # per-example output columns

_nb = ml_dtypes.bfloat16

_cached = {}

# relu-tile engine split by (j*HC+hc) % 14: DVE tensor_scalar ~163ns/tile,
# ACT activation ~292ns/tile -> 9:5 split balances both engines.
RELU_ACT = {0, 3, 6, 9, 12}


def _build_program():
    nc = bacc.Bacc("TRN2", target_bir_lowering=False, debug=False, num_devices=NC_CORES)

    # ---- I/O ----
    ctx_d = nc.dram_tensor("ctx_bf", [NB, 128, DC, 128], BF16, kind="ExternalInput")
    wts_d = nc.dram_tensor("wts_bf", [128, DC * H + 2 * HC * H + 2 + 2 + HC * TAGS], BF16, kind="ExternalInput")
    pkf_d = nc.dram_tensor("pack_f32", [128, 4], F32, kind="ExternalInput")
    gidx_d = nc.dram_tensor("gidx_i", [128, NB], I32, kind="ExternalInput")
    out_d = nc.dram_tensor("logits", [128, NB * OUTW], F32, kind="ExternalOutput")
    cbb_ds = [nc.dram_tensor(f"cbb_scratch{b}", [J, H], F32) for b in range(NB)]

    W1C = DC * H          # w1 cols in wts
    WAC = W1C + HC * H    # wa cols
    WBC = WAC + HC * H    # wb cols
    RTC = WBC             # root at WBC..WBC+2  (interleaved: [root hc0, root hc1])
    ARC = WBC + 2         # w_arc cols (2)
    LBC = WBC + 4         # w_lab cols (HC*TAGS)

    with tile.TileContext(nc) as tc:
        with (
            tc.tile_pool(name="consts", bufs=1) as consts,
            tc.tile_pool(name="bpool", bufs=2) as bpool,
            tc.tile_pool(name="pairs", bufs=48) as pairs_pool,
            tc.tile_pool(name="ps_big", bufs=2, space="PSUM") as ps_big,
            tc.tile_pool(name="ps_work", bufs=2, space="PSUM") as ps_work,
            tc.tile_pool(name="ps_lab", bufs=2, space="PSUM") as ps_lab,
        ):
            # ---- ctx for both examples first (critical path) ----
            ctxTs = []
            ctxT0 = bpool.tile([128, DC, 128], BF16, tag="ctxT")
            nc.sync.dma_start(out=ctxT0[:, 0:2, :], in_=ctx_d.ap()[0, :, 0:2, :])
            nc.scalar.dma_start(out=ctxT0[:, 2:4, :], in_=ctx_d.ap()[0, :, 2:4, :])
            ctxTs.append(ctxT0)
            wts_sb = consts.tile([128, DC * H + 2 * HC * H + 4 + HC * TAGS], BF16)
            # w1 first (needed for hidden), on two queues
            nc.sync.dma_start(out=wts_sb[:, 0 : W1C // 2], in_=wts_d.ap()[:, 0 : W1C // 2])
            nc.gpsimd.dma_start(out=wts_sb[:, W1C // 2 : W1C], in_=wts_d.ap()[:, W1C // 2 : W1C])
            ctxT1 = bpool.tile([128, DC, 128], BF16, tag="ctxT")
            nc.sync.dma_start(out=ctxT1[:], in_=ctx_d.ap()[1])
            # wa, wb, root, w_arc, w_lab
            nc.scalar.dma_start(out=wts_sb[:, W1C:WBC], in_=wts_d.ap()[:, W1C:WBC])
            nc.sync.dma_start(out=wts_sb[:, WBC:], in_=wts_d.ap()[:, WBC:])
            ctxTs.append(ctxT1)
            pkf_sb = consts.tile([128, 4], F32)
            nc.gpsimd.dma_start(out=pkf_sb[:], in_=pkf_d.ap())
            gidx_sb = consts.tile([128, NB], I32)
            nc.gpsimd.dma_start(out=gidx_sb[:], in_=gidx_d.ap())
            ident_sb = consts.tile([128, 128], F32)
            make_identity(nc, ident_sb[:])
            out_sb = consts.tile([128, NB * OUTW], F32)

            def w1_ap(dc, hc):
                return wts_sb[:, dc * H + hc * 128 : dc * H + (hc + 1) * 128]

            def wa_ap(hc, ac):
                return wts_sb[:, W1C + hc * H + ac * 128 : W1C + hc * H + (ac + 1) * 128]

            def wb_ap(hc, bc):
                return wts_sb[:, WAC + hc * H + bc * 128 : WAC + hc * H + (bc + 1) * 128]

            def wb_full(hc):
                return wts_sb[:, WAC + hc * H : WAC + (hc + 1) * H]

            for b in range(NB):
                ctxT = ctxTs[b]
                # ---- hidden (into cwr cols 1..128) ----
                cwrT = bpool.tile([128, HC, J], BF16, tag="cwrT")
                for hc in range(HC):
                    nc.vector.tensor_copy(cwrT[:, hc, 0:1], wts_sb[:, RTC + hc : RTC + hc + 1])
                for hc in range(HC):
                    phw = ps_work.tile([128, H], F32, tag="work")
                    ph = phw[:, :128]
                    for dc in range(DC):
                        nc.tensor.matmul(
                            ph[:],
                            lhsT=w1_ap(dc, hc),
                            rhs=ctxT[:, dc, :],
                            start=(dc == 0),
                            stop=(dc == DC - 1),
                        )
                    nc.vector.tensor_scalar(
                        out=cwrT[:, hc, 1:J],
                        in0=ph[:],
                        scalar1=pkf_sb[:, hc : hc + 1],
                        scalar2=0.0,
                        op0=ALU.add,
                        op1=ALU.max,
                    )
                # ---- ha_T (+bp folded) ----
                haT = bpool.tile([128, HC, 128], BF16, tag="haT")
                for ac in range(HC):
                    paw = ps_work.tile([128, H], F32, tag="work")
                    pa = paw[:, :128]
                    for hc in range(HC):
                        nc.tensor.matmul(
                            pa[:],
                            lhsT=wa_ap(hc, ac),
                            rhs=cwrT[:, hc, 1:J],
                            start=(hc == 0),
                            stop=(hc == HC - 1),
                        )
                    nc.vector.tensor_scalar(
                        out=haT[:, ac, :],
                        in0=pa[:],
                        scalar1=pkf_sb[:, 2 + ac : 3 + ac],
                        scalar2=None,
                        op0=ALU.add,
                    )
                # ---- cbb_T [128, 2, 129] f32 ----
                cbbT = bpool.tile([128, HC, J], F32, tag="cbbT")
                for bc in range(HC):
                    pc = ps_big.tile([128, J], F32, tag="pcb")
                    for hc in range(HC):
                        nc.tensor.matmul(
                            pc[:],
                            lhsT=wb_ap(hc, bc),
                            rhs=cwrT[:, hc, :],
                            start=(hc == 0),
                            stop=(hc == HC - 1),
                        )
                    nc.scalar.copy(cbbT[:, bc, :], pc[:])
                # ---- cbb in [j, h] layout -> DRAM (for the gather) ----
                cj = bpool.tile([128, H], F32, tag="cj")
                pj = ps_work.tile([128, H], F32, tag="work")
                for hc in range(HC):
                    nc.tensor.matmul(
                        pj[:],
                        lhsT=cwrT[:, hc, 0:128],
                        rhs=wb_full(hc),
                        start=(hc == 0),
                        stop=(hc == HC - 1),
                    )
                nc.scalar.copy(cj[:], pj[:])
                st1 = nc.sync.dma_start(out=cbb_ds[b].ap()[0:128, :], in_=cj[:])
                cjl = bpool.tile([1, H], F32, tag="cjl")
                pjlw = ps_work.tile([128, H], F32, tag="work")
                pjl = pjlw[0:1, :]
                for hc in range(HC):
                    nc.tensor.matmul(
                        pjl[:],
                        lhsT=cwrT[:, hc, 128:129],
                        rhs=wb_full(hc),
                        start=(hc == 0),
                        stop=(hc == HC - 1),
                    )
                nc.scalar.copy(cjl[:], pjl[:])
                st2 = nc.sync.dma_start(out=cbb_ds[b].ap()[128:J, :], in_=cjl[:])
                # ---- gather cbb rows at gold arcs ----
                csel = bpool.tile([128, H], F32, tag="csel")
                g = nc.gpsimd.indirect_dma_start(
                    out=csel[:],
                    out_offset=None,
                    in_=cbb_ds[b].ap(),
                    in_offset=IndirectOffsetOnAxis(ap=gidx_sb[:, b : b + 1], axis=0),
                )
                add_dep_helper(g.ins, st1.ins, sync=True, reason="cbb store->gather")
                add_dep_helper(g.ins, st2.ins, sync=True, reason="cbb store->gather")

                # ---- label path ----
                selT = bpool.tile([128, HC, 128], BF16, tag="selT")
                for hc in range(HC):
                    ptrw = ps_work.tile([128, H], F32, tag="work")
                    ptr = ptrw[:, :128]
                    nc.tensor.transpose(
                        ptr[:], csel[:, hc * 128 : (hc + 1) * 128], ident_sb[:]
                    )
                    tmp = bpool.tile([128, 128], F32, tag="seltmp")
                    nc.vector.tensor_add(tmp[:], ptr[:], haT[:, hc, :])
                    nc.vector.tensor_scalar(
                        out=selT[:, hc, :], in0=tmp[:], scalar1=0.0, op0=ALU.max,
                        scalar2=None,
                    )
                lab_ps = ps_lab.tile([128, TAGS], F32, tag="lab")
                for hc in range(HC):
                    nc.tensor.matmul(
                        lab_ps[:],
                        lhsT=selT[:, hc, :],
                        rhs=wts_sb[:, LBC + TAGS * hc : LBC + TAGS * (hc + 1)],
                        start=(hc == 0),
                        stop=(hc == HC - 1),
                    )

                # ---- the quadratic j-loop ----
                arc_ps = ps_big.tile([128, J], F32, tag="arc")
                for j in range(J):
                    for hc in range(HC):
                        pt = pairs_pool.tile([128, 128], BF16, tag="pairs")
                        k = (j * HC + hc) % 14
                        if k in RELU_ACT:
                            nc.scalar.activation(
                                pt[:],
                                haT[:, hc, :],
                                AF.Relu,
                                bias=cbbT[:, hc, j : j + 1],
                            )
                        else:
                            nc.vector.tensor_scalar(
                                out=pt[:],
                                in0=haT[:, hc, :],
                                scalar1=cbbT[:, hc, j : j + 1],
                                scalar2=0.0,
                                op0=ALU.add,
                                op1=ALU.max,
                            )
                        nc.tensor.matmul(
                            arc_ps[:, j : j + 1],
                            lhsT=pt[:],
                            rhs=wts_sb[:, ARC + hc : ARC + hc + 1],
                            start=(hc == 0),
                            stop=(hc == HC - 1),
                        )

                # ---- ship raw logits for this example ----
                ob = b * OUTW
                nc.vector.tensor_copy(out_sb[:, ob : ob + J], arc_ps[:])
                nc.vector.tensor_copy(out_sb[:, ob + J : ob + OUTW], lab_ps[:])
                nc.sync.dma_start(
                    out=out_d.ap()[:, ob : ob + OUTW], in_=out_sb[:, ob : ob + OUTW]
                )

    nc.compile()
    return nc


def _prep_in_maps(inputs):
    ctx = np.asarray(inputs["contextualized"], np.float32)
    arcs = np.asarray(inputs["desired_arcs"], np.int32)
    W1 = np.asarray(inputs["W1"], np.float32)
    b1 = np.asarray(inputs["b1"], np.float32)
    root = np.asarray(inputs["root"], np.float32)
    Wp = np.asarray(inputs["Wp"], np.float32)
    bp = np.asarray(inputs["bp"], np.float32)
    W_arc = np.asarray(inputs["W_arc"], np.float32)
    W_lab = np.asarray(inputs["W_lab"], np.float32)

    def chunked(w, nch):  # [nch*128, X] -> [128, nch*X] col-blocks per chunk
        return w.reshape(nch, 128, -1).transpose(1, 0, 2).reshape(128, -1)

    # wts layout: [w1 (DC*H) | wa (HC*H) | wb (HC*H) | root (2) | w_arc (2) | w_lab (HC*TAGS)]
    w1_bf = chunked(W1, DC)
    wa_bf = chunked(Wp[:H], HC)
    wb_bf = chunked(Wp[H:], HC)
    root_c = root.reshape(HC, 128).T  # [128, 2]
    warc_c = W_arc[:, 0].reshape(HC, 128).T  # [128, 2]
    wlab_c = np.concatenate([W_lab[hc * 128 : (hc + 1) * 128] for hc in range(HC)], axis=1)
    wts = np.concatenate([w1_bf, wa_bf, wb_bf, root_c, warc_c, wlab_c], axis=1).astype(_nb)

    pkf = np.zeros((128, 4), np.float32)
    pkf[:, 0:2] = b1.reshape(HC, 128).T
    pkf[:, 2:4] = bp.reshape(HC, 128).T

    in_maps = []
    for c in range(NC_CORES):
        bs = slice(c * NB, (c + 1) * NB)
        in_maps.append(
            {
                "ctx_bf": np.ascontiguousarray(
                    ctx[bs].reshape(NB, L, DC, 128).transpose(0, 3, 2, 1)
                ).astype(_nb),
                "wts_bf": wts,
                "pack_f32": pkf,
                "gidx_i": np.ascontiguousarray(arcs[bs].T).astype(np.int32),
            }
        )
    return in_maps


def kernel(**inputs) -> np.ndarray:
    if "nc" not in _cached:
        _cached["nc"] = _build_program()
    nc = _cached["nc"]
    in_maps = _prep_in_maps(inputs)
    res = run_bass_kernel_spmd(nc, in_maps, list(range(NC_CORES)))
    # logits: [128 tokens, NB*(J+TAGS)] per core
    outs = [r["logits"] for r in res.results]

    arcs = np.asarray(inputs["desired_arcs"], np.int64)  # [B, L]
    labs = np.asarray(inputs["desired_labels"], np.int64)  # [B, L]
    b_lab = np.asarray(inputs["b_lab"], np.float64)  # [TAGS]
    lens = np.asarray(inputs["sentence_lengths"], np.int32)  # [B]

    total = 0.0
    for c in range(NC_CORES):
        o = outs[c].astype(np.float64)
        for b in range(NB):
            ex = c * NB + b
            arc = o[:, b * OUTW : b * OUTW + J]  # [L, J]
            lab = o[:, b * OUTW + J : (b + 1) * OUTW] + b_lab[None, :]  # [L, TAGS]
            m = np.arange(L) < lens[ex]
            arc_m = arc.max(axis=1)
            arc_lse = np.log(np.exp(arc - arc_m[:, None]).sum(axis=1)) + arc_m
            arc_ce = arc_lse - arc[np.arange(L), arcs[ex]]
            lab_m = lab.max(axis=1)
            lab_lse = np.log(np.exp(lab - lab_m[:, None]).sum(axis=1)) + lab_m
            lab_ce = lab_lse - lab[np.arange(L), labs[ex]]
            total += float(((arc_ce + lab_ce) * m).sum())
    denom = max(float((np.arange(L)[None, :] < lens[:, None]).sum()), 1.0)
    return np.array(0.5 * total / denom, dtype=np.float32)


# revision 5
# speedup vs baseline: 1.1275x; 1.0687x over previous
"""Trainium2 Bass kernel for nn_ModelDEP (biaffine-ish dependency parser loss).

Contract: kernel(**inputs) takes FULL unsharded numpy inputs (as produced by
reference.setup_inputs()) and returns the FULL output (scalar f32 loss).

Strategy (hardcoded, self-contained):
  - Data parallel over batch: B=16 examples -> 8 cores x 2 examples.
  - Per example, on device:
      hidden_T = relu(W1.T @ ctx_T + b1)            [256h x 128i] (h on partitions)
      cwr_T    = [root | hidden_T]                  [256h x 129j]
      ha_T     = Wa.T @ hidden_T + bp               [256 x 128]   (bp folded here)
      cbb_T    = Wb.T @ cwr_T                       [256 x 129]
      arc[i,j] = W_arc . relu(ha_T[:,i] + cbb_T[:,j])
        - per (j, h-chunk): one fused (add bias, max 0) op -> bf16 [128,128] tile
          split DVE tensor_scalar (163ns) : ACT activation relu+bias (292ns) = 9:5
        - TensorE: lhsT = pairs tile (stationary, bf16 FWL), rhs = W_arc chunk
          -> PSUM column [128i, 1], accumulated over the 2 h-chunks
      label path: cbb rows in [j,h] layout -> internal DRAM; indirect-DMA gather
        rows at gold arcs; PE transpose; sel_T = relu(ha_T + gathered.T);
        label logits = sel_T.T @ W_lab (no b_lab; host adds it).
      Ship raw logits: arc_ps [128,129] + lab_ps [128,45] f32 per example.
  - Host: log-softmax CE for arc+label, gold gather, mask by sentence length,
    global sum, /denom, *0.5.  (Softmax epilogue on host kills the Exp/Ln
    activation-table thrash and the on-chip reduce/exp/ln tail.)
"""

import sys
import numpy as np

for _p in ("/opt/trn_rl_repo", "/root/.axon_site/_ro/trn_rl_repo"):
    if _p not in sys.path:
        sys.path.append(_p)

import ml_dtypes

import concourse.bass as bass
from concourse import bacc
import concourse.mybir as mybir
import concourse.tile as tile
from concourse.bass import IndirectOffsetOnAxis
from concourse.bass_utils import run_bass_kernel_spmd
from concourse.masks import make_identity
from concourse.tile_rust import add_dep_helper

BF16 = mybir.dt.bfloat16
F32 = mybir.dt.float32
I32 = mybir.dt.int32
AF = mybir.ActivationFunctionType
ALU = mybir.AluOpType

B, L, D, H, TAGS = 16, 128, 512, 256, 45
NC_CORES = 8
NB = B // NC_CORES  # examples per core
J = L + 1  # head candidates (root + tokens)
HC = H // 128  # h chunks
DC = D // 128  # d chunks
OUTW = J + TAGS  # per-example output columns

_nb = ml_dtypes.bfloat16

_cached = {}

# relu-tile engine split by (j*HC+hc) % 14: DVE tensor_scalar ~163ns/tile,
# ACT activation ~292ns/tile -> 9:5 split balances both engines.
RELU_ACT = {0, 3, 6, 9, 12}


def _build_program():
    nc = bacc.Bacc("TRN2", target_bir_lowering=False, debug=False, num_devices=NC_CORES)

    # ---- I/O ----
    ctx_d = nc.dram_tensor("ctx_bf", [NB, 128, DC, 128], BF16, kind="ExternalInput")
    wts_d = nc.dram_tensor("wts_bf", [128, DC * H + 2 * HC * H + 2 + 2 + HC * TAGS], BF16, kind="ExternalInput")
    pkf_d = nc.dram_tensor("pack_f32", [128, 4], F32, kind="ExternalInput")
    gidx_d = nc.dram_tensor("gidx_i", [128, NB], I32, kind="ExternalInput")
    out_d = nc.dram_tensor("logits", [128, NB * OUTW], F32, kind="ExternalOutput")
    cbb_ds = [nc.dram_tensor(f"cbb_scratch{b}", [J, H], F32) for b in range(NB)]

    W1C = DC * H          # w1 cols in wts
    WAC = W1C + HC * H    # wa cols
    WBC = WAC + HC * H    # wb cols
    RTC = WBC             # root at WBC..WBC+2  (interleaved: [root hc0, root hc1])
    ARC = WBC + 2         # w_arc cols (2)
    LBC = WBC + 4         # w_lab cols (HC*TAGS)

    with tile.TileContext(nc) as tc:
        with (
            tc.tile_pool(name="consts", bufs=1) as consts,
            tc.tile_pool(name="bpool", bufs=2) as bpool,
            tc.tile_pool(name="pairs", bufs=48) as pairs_pool,
            tc.tile_pool(name="ps_big", bufs=2, space="PSUM") as ps_big,
            tc.tile_pool(name="ps_work", bufs=2, space="PSUM") as ps_work,
            tc.tile_pool(name="ps_lab", bufs=2, space="PSUM") as ps_lab,
        ):
            # ---- ctx for both examples first (critical path) ----
            ctxTs = []
            ctxT0 = bpool.tile([128, DC, 128], BF16, tag="ctxT")
            nc.sync.dma_start(out=ctxT0[:, 0:2, :], in_=ctx_d.ap()[0, :, 0:2, :])
            nc.scalar.dma_start(out=ctxT0[:, 2:4, :], in_=ctx_d.ap()[0, :, 2:4, :])
            ctxTs.append(ctxT0)
            wts_sb = consts.tile([128, DC * H + 2 * HC * H + 4 + HC * TAGS], BF16)
            # w1 first (needed for hidden), on two queues
            nc.sync.dma_start(out=wts_sb[:, 0 : W1C // 2], in_=wts_d.ap()[:, 0 : W1C // 2])
            nc.gpsimd.dma_start(out=wts_sb[:, W1C // 2 : W1C], in_=wts_d.ap()[:, W1C // 2 : W1C])
            ctxT1 = bpool.tile([128, DC, 128], BF16, tag="ctxT")
            nc.sync.dma_start(out=ctxT1[:], in_=ctx_d.ap()[1])
            # wa, wb, root, w_arc, w_lab
            nc.scalar.dma_start(out=wts_sb[:, W1C:WBC], in_=wts_d.ap()[:, W1C:WBC])
            nc.sync.dma_start(out=wts_sb[:, WBC:], in_=wts_d.ap()[:, WBC:])
            ctxTs.append(ctxT1)
            pkf_sb = consts.tile([128, 4], F32)
            nc.gpsimd.dma_start(out=pkf_sb[:], in_=pkf_d.ap())
            gidx_sb = consts.tile([128, NB], I32)
            nc.gpsimd.dma_start(out=gidx_sb[:], in_=gidx_d.ap())
            ident_sb = consts.tile([128, 128], F32)
            make_identity(nc, ident_sb[:])
            out_sb = consts.tile([128, NB * OUTW], F32)

            def w1_ap(dc, hc):
                return wts_sb[:, hc * D + dc * 128 : hc * D + (dc + 1) * 128]

            def wa_ap(hc, ac):
                return wts_sb[:, W1C + hc * H + ac * 128 : W1C + hc * H + (ac + 1) * 128]

            def wb_ap(hc, bc):
                return wts_sb[:, WAC + hc * H + bc * 128 : WAC + hc * H + (bc + 1) * 128]

            def wb_full(hc):
                return wts_sb[:, WAC + hc * H : WAC + (hc + 1) * H]

            cwrTs, haTs, cbbTs, csels = {}, {}, {}, {}
            arc_pss, lab_pss = {}, {}

            def prologue(b):
                ctxT = ctxTs[b]
                # hidden (into cwr cols 1..128)
                cwrT = bpool.tile([128, HC, J], BF16, tag="cwrT")
                for hc in range(HC):
                    nc.vector.tensor_copy(cwrT[:, hc, 0:1], wts_sb[:, RTC + hc : RTC + hc + 1])
                for hc in range(HC):
                    phw = ps_work.tile([128, H], F32, tag="work")
                    ph = phw[:, :128]
                    for dc in range(DC):
                        nc.tensor.matmul(
                            ph[:],
                            lhsT=w1_ap(dc, hc),
                            rhs=ctxT[:, dc, :],
                            start=(dc == 0),
                            stop=(dc == DC - 1),
                        )
                    nc.vector.tensor_scalar(
                        out=cwrT[:, hc, 1:J],
                        in0=ph[:],
                        scalar1=pkf_sb[:, hc : hc + 1],
                        scalar2=0.0,
                        op0=ALU.add,
                        op1=ALU.max,
                    )
                # ha_T (+bp folded)
                haT = bpool.tile([128, HC, 128], BF16, tag="haT")
                for ac in range(HC):
                    paw = ps_work.tile([128, H], F32, tag="work")
                    pa = paw[:, :128]
                    for hc in range(HC):
                        nc.tensor.matmul(
                            pa[:],
                            lhsT=wa_ap(hc, ac),
                            rhs=cwrT[:, hc, 1:J],
                            start=(hc == 0),
                            stop=(hc == HC - 1),
                        )
                    nc.vector.tensor_scalar(
                        out=haT[:, ac, :],
                        in0=pa[:],
                        scalar1=pkf_sb[:, 2 + ac : 3 + ac],
                        scalar2=None,
                        op0=ALU.add,
                    )
                # cbb_T [128, 2, 129] f32
                cbbT = bpool.tile([128, HC, J], F32, tag="cbbT")
                for bc in range(HC):
                    pc = ps_big.tile([128, J], F32, tag="pcb")
                    for hc in range(HC):
                        nc.tensor.matmul(
                            pc[:],
                            lhsT=wb_ap(hc, bc),
                            rhs=cwrT[:, hc, :],
                            start=(hc == 0),
                            stop=(hc == HC - 1),
                        )
                    nc.scalar.copy(cbbT[:, bc, :], pc[:])
                # cbb in [j, h] layout -> DRAM (for the gather)
                cj = bpool.tile([128, H], F32, tag="cj")
                pj = ps_work.tile([128, H], F32, tag="work")
                for hc in range(HC):
                    nc.tensor.matmul(
                        pj[:],
                        lhsT=cwrT[:, hc, 0:128],
                        rhs=wb_full(hc),
                        start=(hc == 0),
                        stop=(hc == HC - 1),
                    )
                nc.scalar.copy(cj[:], pj[:])
                st1 = nc.sync.dma_start(out=cbb_ds[b].ap()[0:128, :], in_=cj[:])
                cjl = bpool.tile([1, H], F32, tag="cjl")
                pjlw = ps_work.tile([128, H], F32, tag="work")
                pjl = pjlw[0:1, :]
                for hc in range(HC):
                    nc.tensor.matmul(
                        pjl[:],
                        lhsT=cwrT[:, hc, 128:129],
                        rhs=wb_full(hc),
                        start=(hc == 0),
                        stop=(hc == HC - 1),
                    )
                nc.scalar.copy(cjl[:], pjl[:])
                st2 = nc.sync.dma_start(out=cbb_ds[b].ap()[128:J, :], in_=cjl[:])
                # gather cbb rows at gold arcs
                csel = bpool.tile([128, H], F32, tag="csel")
                g = nc.gpsimd.indirect_dma_start(
                    out=csel[:],
                    out_offset=None,
                    in_=cbb_ds[b].ap(),
                    in_offset=IndirectOffsetOnAxis(ap=gidx_sb[:, b : b + 1], axis=0),
                )
                add_dep_helper(g.ins, st1.ins, sync=True, reason="cbb store->gather")
                add_dep_helper(g.ins, st2.ins, sync=True, reason="cbb store->gather")
                cwrTs[b], haTs[b], cbbTs[b], csels[b] = cwrT, haT, cbbT, csel

            def label(b):
                haT, csel = haTs[b], csels[b]
                selT = bpool.tile([128, HC, 128], BF16, tag="selT")
                for hc in range(HC):
                    ptrw = ps_work.tile([128, H], F32, tag="work")
                    ptr = ptrw[:, :128]
                    nc.tensor.transpose(
                        ptr[:], csel[:, hc * 128 : (hc + 1) * 128], ident_sb[:]
                    )
                    tmp = bpool.tile([128, 128], F32, tag="seltmp")
                    nc.vector.tensor_add(tmp[:], ptr[:], haT[:, hc, :])
                    nc.vector.tensor_scalar(
                        out=selT[:, hc, :], in0=tmp[:], scalar1=0.0, op0=ALU.max,
                        scalar2=None,
                    )
                lab_ps = ps_lab.tile([128, TAGS], F32, tag="lab")
                for hc in range(HC):
                    nc.tensor.matmul(
                        lab_ps[:],
                        lhsT=selT[:, hc, :],
                        rhs=wts_sb[:, LBC + TAGS * hc : LBC + TAGS * (hc + 1)],
                        start=(hc == 0),
                        stop=(hc == HC - 1),
                    )
                lab_pss[b] = lab_ps

            def jloop(b):
                haT, cbbT = haTs[b], cbbTs[b]
                arc_ps = ps_big.tile([128, J], F32, tag="arc")
                for j in range(J):
                    for hc in range(HC):
                        pt = pairs_pool.tile([128, 128], BF16, tag="pairs")
                        k = (j * HC + hc) % 14
                        if k in RELU_ACT:
                            nc.scalar.activation(
                                pt[:],
                                haT[:, hc, :],
                                AF.Relu,
                                bias=cbbT[:, hc, j : j + 1],
                            )
                        else:
                            nc.vector.tensor_scalar(
                                out=pt[:],
                                in0=haT[:, hc, :],
                                scalar1=cbbT[:, hc, j : j + 1],
                                scalar2=0.0,
                                op0=ALU.add,
                                op1=ALU.max,
                            )
                        nc.tensor.matmul(
                            arc_ps[:, j : j + 1],
                            lhsT=pt[:],
                            rhs=wts_sb[:, ARC + hc : ARC + hc + 1],
                            start=(hc == 0),
                            stop=(hc == HC - 1),
                        )
                arc_pss[b] = arc_ps

            def out_ex(b):
                ob = b * OUTW
                nc.vector.tensor_copy(out_sb[:, ob : ob + J], arc_pss[b][:])
                nc.vector.tensor_copy(out_sb[:, ob + J : ob + OUTW], lab_pss[b][:])
                nc.sync.dma_start(
                    out=out_d.ap()[:, ob : ob + OUTW], in_=out_sb[:, ob : ob + OUTW]
                )

            prologue(0)
            prologue(1)
            jloop(0)
            label(0)
            label(1)
            out_ex(0)
            jloop(1)
            out_ex(1)

    nc.compile()
    return nc


def _prep_in_maps(inputs):
    ctx = np.asarray(inputs["contextualized"], np.float32)
    arcs = np.asarray(inputs["desired_arcs"], np.int32)
    W1 = np.asarray(inputs["W1"], np.float32)
    b1 = np.asarray(inputs["b1"], np.float32)
    root = np.asarray(inputs["root"], np.float32)
    Wp = np.asarray(inputs["Wp"], np.float32)
    bp = np.asarray(inputs["bp"], np.float32)
    W_arc = np.asarray(inputs["W_arc"], np.float32)
    W_lab = np.asarray(inputs["W_lab"], np.float32)

    def chunked(w, nch):  # [nch*128, X] -> [128, nch*X] col-blocks per chunk
        return w.reshape(nch, 128, -1).transpose(1, 0, 2).reshape(128, -1)

    # wts layout: [w1 (DC*H, hc-major: col hc*D+dc*128) | wa | wb | root | w_arc | w_lab]
    w1_bf = np.concatenate(
        [chunked(W1[:, hc * 128 : (hc + 1) * 128], DC) for hc in range(HC)], axis=1
    )
    wa_bf = chunked(Wp[:H], HC)
    wb_bf = chunked(Wp[H:], HC)
    root_c = root.reshape(HC, 128).T  # [128, 2]
    warc_c = W_arc[:, 0].reshape(HC, 128).T  # [128, 2]
    wlab_c = np.concatenate([W_lab[hc * 128 : (hc + 1) * 128] for hc in range(HC)], axis=1)
    wts = np.concatenate([w1_bf, wa_bf, wb_bf, root_c, warc_c, wlab_c], axis=1).astype(_nb)

    pkf = np.zeros((128, 4), np.float32)
    pkf[:, 0:2] = b1.reshape(HC, 128).T
    pkf[:, 2:4] = bp.reshape(HC, 128).T

    in_maps = []
    for c in range(NC_CORES):
        bs = slice(c * NB, (c + 1) * NB)
        in_maps.append(
            {
                "ctx_bf": np.ascontiguousarray(
                    ctx[bs].reshape(NB, L, DC, 128).transpose(0, 3, 2, 1)
                ).astype(_nb),
                "wts_bf": wts,
                "pack_f32": pkf,
                "gidx_i": np.ascontiguousarray(arcs[bs].T).astype(np.int32),
            }
        )
    return in_maps


def kernel(**inputs) -> np.ndarray:
    if "nc" not in _cached:
        _cached["nc"] = _build_program()
    nc = _cached["nc"]
    in_maps = _prep_in_maps(inputs)
    res = run_bass_kernel_spmd(nc, in_maps, list(range(NC_CORES)))
    # logits: [128 tokens, NB*(J+TAGS)] per core
    outs = [r["logits"] for r in res.results]

    arcs = np.asarray(inputs["desired_arcs"], np.int64)  # [B, L]
    labs = np.asarray(inputs["desired_labels"], np.int64)  # [B, L]
    b_lab = np.asarray(inputs["b_lab"], np.float64)  # [TAGS]
    lens = np.asarray(inputs["sentence_lengths"], np.int32)  # [B]

    total = 0.0
    for c in range(NC_CORES):
        o = outs[c].astype(np.float64)
        for b in range(NB):
            ex = c * NB + b
            arc = o[:, b * OUTW : b * OUTW + J]  # [L, J]
            lab = o[:, b * OUTW + J : (b + 1) * OUTW] + b_lab[None, :]  # [L, TAGS]
            m = np.arange(L) < lens[ex]
            arc_m = arc.max(axis=1)
            arc_lse = np.log(np.exp(arc - arc_m[:, None]).sum(axis=1)) + arc_m
            arc_ce = arc_lse - arc[np.arange(L), arcs[ex]]
            lab_m = lab.max(axis=1)
            lab_lse = np.log(np.exp(lab - lab_m[:, None]).sum(axis=1)) + lab_m
            lab_ce = lab_lse - lab[np.arange(L), labs[ex]]
            total += float(((arc_ce + lab_ce) * m).sum())
    denom = max(float((np.arange(L)[None, :] < lens[:, None]).sum()), 1.0)
    return np.array(0.5 * total / denom, dtype=np.float32)
